# revision 18
# baseline (speedup 1.0000x reference)
"""GAT neighbor-aggregation kernel for Trainium2, 8-core data-parallel.

Math (per batch b):
  vu = ea @ U2 ; iv = ea @ W2
  logits[i,j] = sum_c yita_c * leaky_relu(vu[i,c] + iv[j,c], 0.2)
  alpha = softmax_j(where(adj>0, logits, -1e12))
  out = leaky_relu(alpha @ ea, 0.2)

Kernel decomposition used on device:
  leaky_relu(v) = 0.8*relu(v) + 0.2*v
  logits[i,j] = 0.2*p_i + 0.2*q_j + sum_c (0.8*sign(yita_c)) * relu(s[i,c] + t[j,c])
  with s = vu * |yita|, t = iv * |yita|; p_i dropped (softmax row constant);
  exp(0.2 q_j) folded multiplicatively into the final alpha @ ea matmul by
  pre-scaling ea rows.  All O(e*c) setup (sPair, tT2, eaS, mask) is
  precomputed on the host; the device does only the O(e^2*c) work:
    - pairwise relu(s_i + t_j) tiles in fp16, [c-pair, j] layout (2 i's in
      128 partitions), split between the vector engine (fused tensor_scalar
      add+max, 2x fp16, ~350ns/tile) and the scalar engine (Relu with
      per-partition bias, ~620ns/tile) by an explicit per-pair schedule that
      keeps the scalar engine free around its exp bursts,
    - c-reduction on the tensor engine with one-hot +-0.8 sign weights,
      round-robin over the four 32-row PSUM column groups,
    - adjacency mask folded into the same PSUM accumulation as one
      full-width matmul adding -60000 at masked (i,j) (identity lhsT),
    - softmax without max-subtraction (|logits| < 8 so fp16 exp is safe);
      exp emits the row-sum denominator via accum_out; tile 0's alpha
      transposes go through the DMA xbar (idle engines), tile 1's through
      the PE + vector-engine copies for a short tail,
    - out = Prelu(P * 1/denom, 0.2) in fp16.

Sharding: core = 2*b + h handles batch b, query rows i in [256h, 256h+256).
"""

import numpy as np
from contextlib import ExitStack

import concourse.bass as bass
import concourse.tile as tile
from concourse import bacc, mybir
from concourse.bass_utils import run_bass_kernel_spmd

F32 = mybir.dt.float32
F16 = mybir.dt.float16
OP = mybir.AluOpType

BSZ, E, C = 4, 512, 64
NCORE = 8
IPC = E // 2          # 256 query rows per core
NPAIR = IPC // 2      # 128 i-pairs per core
NTILE = IPC // 128    # 2 logits tiles of 128 i-rows
N_WARM = 72           # PE warmup matmuls issued while input DMAs are in flight
MASKV = -60000.0      # mask add value; exp(-60000) == 0 in fp16/fp32


def _issue_schedule():
    """Engine per ISSUE position: True = scalar engine (ACT), False = DVE.

    Production happens in issue order (the PE consumes x tiles in order), so
    the balance must be built in issue space.  Greedy: assign each tile to
    the engine that finishes it soonest, modeling sustained rates (DVE
    263ns/tile, ACT 620ns/tile), the ACT table load (1283ns) and the tile-0
    exp burst (~1000ns injected at issue 64).  ACT is barred from issue
    60..73 (fast tile-0 close + exp burst) and the last 6 (fast tail +
    tile-1 exps).
    """
    sched = [False] * NPAIR
    dve, act = 0.0, 1283.0
    for i in range(NPAIR):
        if i == 64:
            act += 1000.0
        if (60 <= i < 74 or i >= NPAIR - 6) or act + 620.0 > dve + 263.0:
            dve += 263.0
        else:
            sched[i] = True
            act += 620.0
    return sched


SCHED = _issue_schedule()


def _build_program():
    nc = bacc.Bacc(
        "TRN2",
        target_bir_lowering=False,
        debug=False,
        enable_asserts=False,
        num_devices=NCORE,
    )
    tT2_ap = nc.dram_tensor("tT2", [128, E], F16, kind="ExternalInput").ap()
    sPair_ap = nc.dram_tensor("sPair", [128, NPAIR], F32, kind="ExternalInput").ap()
    whot_ap = nc.dram_tensor("whot", [128, 2048], F16, kind="ExternalInput").ap()
    wident_ap = nc.dram_tensor("wident", [128, 128], F16, kind="ExternalInput").ap()
    eaS_ap = nc.dram_tensor("eaS", [128, 4 * (C + 1)], F16, kind="ExternalInput").ap()
    madj_ap = nc.dram_tensor("madj", [128, NTILE * E], F16, kind="ExternalInput").ap()
    out_ap = nc.dram_tensor("out", [IPC, C], F16, kind="ExternalOutput").ap()

    with tile.TileContext(nc) as tc:
        with ExitStack() as ctx:
            singles = ctx.enter_context(tc.tile_pool(name="singles", bufs=1))
            xpool = ctx.enter_context(tc.tile_pool(name="xpool", bufs=24))
            ps_logits = ctx.enter_context(
                tc.tile_pool(name="ps_logits", bufs=2, space="PSUM")
            )
            ps_tp = ctx.enter_context(tc.tile_pool(name="ps_tp", bufs=2, space="PSUM"))
            ps_fm = ctx.enter_context(tc.tile_pool(name="ps_fm", bufs=2, space="PSUM"))
            small = ctx.enter_context(tc.tile_pool(name="small", bufs=6))
            epool = ctx.enter_context(tc.tile_pool(name="epool", bufs=4))
            atpool = ctx.enter_context(tc.tile_pool(name="atpool", bufs=4))

            # ---- PE warmup: no input deps, runs during the DMA fill ----
            warm_sb = singles.tile([128, C], F16, tag="warm")
            nc.vector.memset(warm_sb[:], 0.0)
            warm_ps = ps_fm.tile([C, C], F32, tag="fm")
            for _ in range(N_WARM):
                nc.tensor.matmul(warm_ps[:], lhsT=warm_sb[:, 0:C], rhs=warm_sb[:])

            # ---- input DMAs: issue in parallel across the three DMA-capable
            # queues; the tensors gating the pairwise loop (tT2, sPair) and
            # the big whot go first on their queues ----
            tT2 = singles.tile([128, E], F16, tag="tT2")
            nc.sync.dma_start(tT2[:], tT2_ap[:])
            sPair = singles.tile([128, NPAIR], F32, tag="sPair")
            nc.scalar.dma_start(sPair[:], sPair_ap[:])
            whot_sb = singles.tile([128, 2048], F16, tag="whot")
            nc.gpsimd.dma_start(whot_sb[:], whot_ap[:])
            madj_sb = singles.tile([128, NTILE, E], F16, tag="madj")
            nc.sync.dma_start(madj_sb[:], madj_ap.rearrange("p (t j) -> p t j", t=NTILE))
            ident_sb = singles.tile([128, 128], F16, tag="ident")
            nc.gpsimd.dma_start(ident_sb[:], wident_ap[:])
            eaS = singles.tile([128, 4, C + 1], F16, tag="eaS")
            nc.gpsimd.dma_start(eaS[:], eaS_ap.rearrange("p (ch c) -> p ch c", ch=4))

            # ---- main: per 128-row logits tile ----
            for t in range(NTILE):
                logits_ps = ps_logits.tile([128, E], F32, tag="logits")
                # round-robin over the four 32-row PSUM column groups so
                # consecutive matmuls hit disjoint PE column groups; each
                # (kk, g) uses a distinct lhsT address to force a real
                # LDWEIGHTS into that column group
                for kk in range(16):
                    for g in range(4):
                        p = t * 64 + g * 16 + kk  # global pair index
                        x = xpool.tile([128, E], F16, tag="x")
                        if SCHED[t * 64 + kk * 4 + g]:
                            nc.scalar.activation(
                                x[:], tT2[:], mybir.ActivationFunctionType.Relu,
                                bias=sPair[:, p : p + 1], scale=1.0,
                            )
                        else:
                            nc.vector.tensor_scalar(
                                x[:], tT2[:], sPair[:, p : p + 1], 0.0, OP.add, OP.max
                            )
                        v = kk * 4 + g
                        nc.tensor.matmul(
                            logits_ps[32 * g : 32 * g + 32, :],
                            lhsT=whot_sb[:, 32 * v : 32 * v + 32],
                            rhs=x[:],
                            start=(kk == 0),
                            stop=False,
                            tile_position=(0, 32 * g),
                        )
                # mask + column bias: logits += -60000 * (1 - adj) + 0.2*q_j,
                # one full-width matmul (identity lhsT) closing the group
                nc.tensor.matmul(
                    logits_ps[:],
                    lhsT=ident_sb[:],
                    rhs=madj_sb[:, t, :],
                    start=False,
                    stop=True,
                    skip_group_check=True,
                )
                # softmax numerator (no max-sub) per 128-col chunk; the ones
                # column of eaS yields the denominator through the fm matmul
                fm_ps = ps_fm.tile([128, C + 1], F32, tag="fm")
                for ch in range(4):
                    e_h = epool.tile([128, 128], F16, tag="esb")
                    nc.scalar.activation(
                        e_h[:], logits_ps[:, ch * 128 : (ch + 1) * 128],
                        mybir.ActivationFunctionType.Exp, bias=0.0, scale=1.0,
                    )
                    aT = atpool.tile([128, 128], F16, tag="aT")
                    if t == 0 and not int(__import__("os").environ.get("NODMAT", "0")):
                        # mid-kernel: transpose via the DMA xbar (idle)
                        nc.sync.dma_start_transpose(aT[:], e_h[:])
                    else:
                        # tail: PE transpose + vector-engine copy (fast path)
                        tp = ps_tp.tile([128, 128], F16, tag="tp")
                        nc.tensor.transpose(tp[:], e_h[:], ident_sb)
                        nc.vector.tensor_copy(aT[:], tp[:])
                    nc.tensor.matmul(
                        fm_ps[:],
                        lhsT=aT[:],
                        rhs=eaS[:, ch, :],
                        start=(ch == 0),
                        stop=(ch == 3),
                    )
                # out = leaky_relu(P / denom) = prelu(P * rec, 0.2), rec > 0
                rec = small.tile([128, 1], F32, tag="rec")
                nc.vector.reciprocal(rec[:], fm_ps[:, C : C + 1])
                out_sb = small.tile([128, C], F16, tag="outsb")
                nc.scalar.activation(
                    out_sb[:], fm_ps[:, 0:C], mybir.ActivationFunctionType.Prelu,
                    bias=0.0, scale=rec[:], alpha=0.2,
                )
                nc.sync.dma_start(out_ap[t * 128 : (t + 1) * 128, :], out_sb[:])

    nc.finalize()
    return nc


_NC = None


def _get_nc():
    global _NC
    if _NC is None:
        _NC = _build_program()
    return _NC


def _host_prep(edge_attr, edge_adj, W_2, U_2, yita):
    edge_attr = np.asarray(edge_attr, dtype=np.float32)
    edge_adj = np.asarray(edge_adj)
    W_2 = np.asarray(W_2, dtype=np.float32)
    U_2 = np.asarray(U_2, dtype=np.float32)
    yita = np.asarray(yita, dtype=np.float32)

    y = yita[:, 0]
    ay = np.abs(y)
    w08 = (0.8 * np.sign(y)).astype(np.float16)
    whot = np.zeros((128, 2048), dtype=np.float16)
    for kk in range(16):
        for g in range(4):
            v = kk * 4 + g
            whot[0:C, 32 * v + 2 * kk] = w08
            whot[C:128, 32 * v + 2 * kk + 1] = w08
    wident = np.eye(128, dtype=np.float16)

    in_maps = []
    for core in range(NCORE):
        b, h = divmod(core, 2)
        ea = edge_attr[b]                      # [E, C]
        vu = ea @ U_2                          # [E, C]
        iv = ea @ W_2                          # [E, C]
        s = vu * ay[None, :]                   # [E, C]
        t = iv * ay[None, :]                   # [E, C]
        q = iv @ y                             # [E]
        eq = np.exp(0.2 * q)                   # [E]

        # sPair[:, p]: rows 0:64 = s[i=2p], rows 64:128 = s[2p+1] (local i)
        sh = s[h * IPC : (h + 1) * IPC]        # [IPC, C]
        sPair = np.empty((128, NPAIR), dtype=np.float32)
        sPair[0:C, :] = sh[0::2].T
        sPair[C:128, :] = sh[1::2].T

        # tT2: [c, j] duplicated into both partition halves
        tT2 = np.empty((128, E), dtype=np.float16)
        tT2[0:C, :] = t.T.astype(np.float16)
        tT2[C:128, :] = tT2[0:C, :]

        # eaS[:, ch, 0:C] = ea in chunk layout, col C = 1 (denominator); the
        # 0.2*q_j column bias rides the mask matmul so exp already carries it
        eaS = np.empty((128, 4, C + 1), dtype=np.float16)
        for chn in range(4):
            rows = slice(chn * 128, (chn + 1) * 128)
            eaS[:, chn, 0:C] = ea[rows].astype(np.float16)
            eaS[:, chn, C] = np.float16(1.0)

        # madj[r, t, j] = (MASKV if adj == 0 else 0) + 0.2*q_j,
        # for i = h*IPC + t*128 + r
        adjh = edge_adj[b, h * IPC : (h + 1) * IPC, :]  # [IPC, E]
        madj = np.where(adjh > 0, 0.0, MASKV) + 0.2 * q[None, :]
        madj = madj.astype(np.float16)
        madj = madj.reshape(NTILE, 128, E).transpose(1, 0, 2)  # [128, NTILE, E]

        in_maps.append(
            {
                "tT2": tT2,
                "sPair": sPair,
                "whot": whot,
                "wident": wident,
                "eaS": np.ascontiguousarray(eaS.reshape(128, 4 * (C + 1))),
                "madj": np.ascontiguousarray(madj.reshape(128, NTILE * E)),
            }
        )
    return in_maps


def kernel(edge_attr, edge_adj, e_max=None, mask=None, W_2=None, U_2=None, yita=None):
    nc = _get_nc()
    in_maps = _host_prep(edge_attr, edge_adj, W_2, U_2, yita)
    res = run_bass_kernel_spmd(nc, in_maps, core_ids=list(range(NCORE)))
    out = np.empty((BSZ, E, C), dtype=np.float32)
    for core in range(NCORE):
        b, h = divmod(core, 2)
        out[b, h * IPC : (h + 1) * IPC, :] = res.results[core]["out"].astype(
            np.float32
        )
    return out


# revision 19
# speedup vs baseline: 1.4514x; 1.4514x over previous
"""GAT neighbor-aggregation kernel for Trainium2, 8-core data-parallel.

Math (per batch b):
  vu = ea @ U2 ; iv = ea @ W2
  logits[i,j] = sum_c yita_c * leaky_relu(vu[i,c] + iv[j,c], 0.2)
  alpha = softmax_j(where(adj>0, logits, -1e12))
  out = leaky_relu(alpha @ ea, 0.2)

Device decomposition (quantized-interpolation table matmul):
  leaky_relu(v) = 0.8*relu(v) + 0.2*v, so with s = vu*|yita|, t = iv*|yita|,
  w_c = 0.8*sign(yita_c):
    logits[i,j] = 0.2*p_i + 0.2*q_j + sum_c w_c * relu(s[i,c] + t[j,c])
  (p_i dropped: softmax row constant).  relu(s+t) is piecewise linear in s,
  so with per-(core,c) levels sigma_{c,0..L-1} spanning [min_i s, max_i s]:
    relu(s_ic + t_jc) = (1-u)*relu(sigma_l + t_jc) + u*relu(sigma_{l+1} + t_jc)
  EXACTLY unless the cell straddles the kink -t_jc (error <= cell/4 there;
  measured end-to-end rel error ~5e-3 at L=24, budget 2e-2).  Therefore
    R[i,j] ~= sum_{(c,l)} W[(c,l),i] * T[(c,l),j]
  one dense matmul with contraction K = 64*L = 1536 (12 chained 128-row
  matmuls per 128-i output tile).  W (interpolation weights, 2 nonzeros per
  c per column) and T (tables relu(sigma+t)) are host-precomputed from the
  O(e*c) quantities; the O(e^2) work runs on the tensor engine at full
  128-wide utilization instead of being elementwise-bound.

  The adjacency mask (+ the 0.2*q_j column bias) is one more accumulated
  matmul adding -60000*(1-adj)+0.2*q_j (identity lhsT, fp16 rhs), so
  exp(masked logit) flushes to 0.  Softmax runs without max-subtraction
  (|logits| < 8, fp16-exp safe): per-128-col chunks, exp (scalar engine,
  PSUM->SBUF fp16) -> transpose (PE) -> copy (vector engine) -> alpha @ eaS
  matmul (PE, with a ones column for the denominator), then
  out = Prelu(P * 1/denom, 0.2) in fp16.

Sharding: core = 2*b + h handles batch b, query rows i in [256h, 256h+256).
"""

import numpy as np
from contextlib import ExitStack

import concourse.bass as bass
import concourse.tile as tile
from concourse import bacc, mybir
from concourse.bass_utils import run_bass_kernel_spmd

F32 = mybir.dt.float32
F16 = mybir.dt.float16
OP = mybir.AluOpType

BSZ, E, C = 4, 512, 64
NCORE = 8
IPC = E // 2          # 256 query rows per core
NTILE = IPC // 128    # 2 logits tiles of 128 i-rows
LVL = 24              # interpolation levels per channel
K = C * LVL           # table contraction size
NCH = K // 128        # 12 contraction chunks of 128
N_WARM = 56           # PE warmup matmuls issued while input DMAs are in flight
MASKV = -60000.0      # mask add value; exp(-60000) == 0 in fp16/fp32


def _build_program():
    nc = bacc.Bacc(
        "TRN2",
        target_bir_lowering=False,
        debug=False,
        enable_asserts=False,
        num_devices=NCORE,
    )
    # T split in 3 parts of 4 chunks so the PE can start on part 1 while
    # parts 2/3 are still on the wire
    tab_aps = [
        nc.dram_tensor(f"tab{i}", [128, 4 * E], F16, kind="ExternalInput").ap()
        for i in range(NCH // 4)
    ]
    w_aps = [
        nc.dram_tensor(f"wint{t}", [128, NCH * 128], F16, kind="ExternalInput").ap()
        for t in range(NTILE)
    ]
    wident_ap = nc.dram_tensor("wident", [128, 128], F16, kind="ExternalInput").ap()
    eaS_ap = nc.dram_tensor("eaS", [128, 4 * (C + 1)], F16, kind="ExternalInput").ap()
    madj_ap = nc.dram_tensor("madj", [128, NTILE * E], F16, kind="ExternalInput").ap()
    out_ap = nc.dram_tensor("out", [IPC, C], F16, kind="ExternalOutput").ap()

    with tile.TileContext(nc) as tc:
        with ExitStack() as ctx:
            singles = ctx.enter_context(tc.tile_pool(name="singles", bufs=1))
            ps_logits = ctx.enter_context(
                tc.tile_pool(name="ps_logits", bufs=2, space="PSUM")
            )
            ps_tp = ctx.enter_context(tc.tile_pool(name="ps_tp", bufs=2, space="PSUM"))
            ps_fm = ctx.enter_context(tc.tile_pool(name="ps_fm", bufs=2, space="PSUM"))
            small = ctx.enter_context(tc.tile_pool(name="small", bufs=6))
            epool = ctx.enter_context(tc.tile_pool(name="epool", bufs=4))
            atpool = ctx.enter_context(tc.tile_pool(name="atpool", bufs=4))

            # ---- PE warmup: no input deps, runs during the DMA fill ----
            warm_sb = singles.tile([128, C], F16, tag="warm")
            nc.vector.memset(warm_sb[:], 0.0)
            warm_ps = ps_fm.tile([C, C], F32, tag="fm")
            for _ in range(N_WARM):
                nc.tensor.matmul(warm_ps[:], lhsT=warm_sb[:, 0:C], rhs=warm_sb[:])

            # ---- input DMAs across the three DMA-capable queues; W0 + the
            # first T part gate the first table matmuls, so they go first ----
            wint = []
            for t in range(NTILE):
                wt = singles.tile([128, NCH, 128], F16, tag=f"wint{t}")
                nc.scalar.dma_start(
                    wt[:], w_aps[t].rearrange("p (q m) -> p q m", q=NCH)
                )
                wint.append(wt)
            tabs = []
            for i in range(NCH // 4):
                tb = singles.tile([128, 4, E], F16, tag=f"tab{i}")
                eng = nc.sync if i < 2 else nc.gpsimd
                eng.dma_start(tb[:], tab_aps[i].rearrange("p (q j) -> p q j", q=4))
                tabs.append(tb)
            madj_sb = singles.tile([128, NTILE, E], F16, tag="madj")
            nc.gpsimd.dma_start(
                madj_sb[:], madj_ap.rearrange("p (t j) -> p t j", t=NTILE)
            )
            ident_sb = singles.tile([128, 128], F16, tag="ident")
            nc.gpsimd.dma_start(ident_sb[:], wident_ap[:])
            eaS = singles.tile([128, 4, C + 1], F16, tag="eaS")
            nc.gpsimd.dma_start(eaS[:], eaS_ap.rearrange("p (ch c) -> p ch c", ch=4))

            # ---- logits: 12 chained dense matmuls per 128-i tile ----
            for t in range(NTILE):
                logits_ps = ps_logits.tile([128, E], F32, tag="logits")
                for q in range(NCH):
                    nc.tensor.matmul(
                        logits_ps[:],
                        lhsT=wint[t][:, q, :],
                        rhs=tabs[q // 4][:, q % 4, :],
                        start=(q == 0),
                        stop=False,
                    )
                # mask + column bias: logits += -60000*(1-adj) + 0.2*q_j
                nc.tensor.matmul(
                    logits_ps[:],
                    lhsT=ident_sb[:],
                    rhs=madj_sb[:, t, :],
                    start=False,
                    stop=True,
                    skip_group_check=True,
                )
                # softmax numerator (no max-sub) per 128-col chunk; the ones
                # column of eaS yields the denominator through the fm matmul
                fm_ps = ps_fm.tile([128, C + 1], F32, tag="fm")
                for ch in range(4):
                    e_h = epool.tile([128, 128], F16, tag="esb")
                    nc.scalar.activation(
                        e_h[:], logits_ps[:, ch * 128 : (ch + 1) * 128],
                        mybir.ActivationFunctionType.Exp, bias=0.0, scale=1.0,
                    )
                    tp = ps_tp.tile([128, 128], F16, tag="tp")
                    nc.tensor.transpose(tp[:], e_h[:], ident_sb)
                    aT = atpool.tile([128, 128], F16, tag="aT")
                    nc.vector.tensor_copy(aT[:], tp[:])
                    nc.tensor.matmul(
                        fm_ps[:],
                        lhsT=aT[:],
                        rhs=eaS[:, ch, :],
                        start=(ch == 0),
                        stop=(ch == 3),
                    )
                # out = leaky_relu(P / denom) = prelu(P * rec, 0.2), rec > 0
                rec = small.tile([128, 1], F32, tag="rec")
                nc.vector.reciprocal(rec[:], fm_ps[:, C : C + 1])
                out_sb = small.tile([128, C], F16, tag="outsb")
                nc.scalar.activation(
                    out_sb[:], fm_ps[:, 0:C], mybir.ActivationFunctionType.Prelu,
                    bias=0.0, scale=rec[:], alpha=0.2,
                )
                nc.sync.dma_start(out_ap[t * 128 : (t + 1) * 128, :], out_sb[:])

    nc.finalize()
    return nc


_NC = None


def _get_nc():
    global _NC
    if _NC is None:
        _NC = _build_program()
    return _NC


def _host_prep(edge_attr, edge_adj, W_2, U_2, yita):
    edge_attr = np.asarray(edge_attr, dtype=np.float32)
    edge_adj = np.asarray(edge_adj)
    W_2 = np.asarray(W_2, dtype=np.float32)
    U_2 = np.asarray(U_2, dtype=np.float32)
    yita = np.asarray(yita, dtype=np.float32)

    y = yita[:, 0]
    ay = np.abs(y)
    w = (0.8 * np.sign(y)).astype(np.float32)
    wident = np.eye(128, dtype=np.float16)

    in_maps = []
    for core in range(NCORE):
        b, h = divmod(core, 2)
        ea = edge_attr[b]                      # [E, C]
        vu = ea @ U_2
        iv = ea @ W_2
        s = vu * ay[None, :]                   # [E, C]
        t = iv * ay[None, :]                   # [E, C]
        q = iv @ y                             # [E]

        sh = s[h * IPC : (h + 1) * IPC]        # [IPC, C]
        lo, hi = sh.min(0), sh.max(0)          # [C]
        span = np.maximum(hi - lo, 1e-6)
        # levels [LVL, C], uniform per channel
        sig = lo[None, :] + span[None, :] * (
            np.arange(LVL, dtype=np.float32)[:, None] / (LVL - 1)
        )
        # tables T[(c,l), j] = relu(sig[l,c] + t[j,c]), contraction-major
        T = np.maximum(sig.T[:, :, None] + t.T[:, None, :], 0.0)  # [C, LVL, E]
        T = T.reshape(K, E).astype(np.float16)

        # interpolation weights W[(c,l), i_local]
        delta = span / (LVL - 1)
        u = (sh - lo[None, :]) / delta[None, :]       # [IPC, C]
        l0 = np.clip(np.floor(u).astype(np.int64), 0, LVL - 2)
        frac = (u - l0).astype(np.float32)
        Wt = np.zeros((C, LVL, IPC), dtype=np.float32)
        ii = np.arange(IPC)
        for c in range(C):
            Wt[c, l0[:, c], ii] += w[c] * (1.0 - frac[:, c])
            Wt[c, l0[:, c] + 1, ii] += w[c] * frac[:, c]
        Wt = Wt.reshape(K, IPC).astype(np.float16)

        # eaS[:, ch, 0:C] = ea in chunk layout, col C = 1 (denominator)
        eaS = np.empty((128, 4, C + 1), dtype=np.float16)
        for chn in range(4):
            rows = slice(chn * 128, (chn + 1) * 128)
            eaS[:, chn, 0:C] = ea[rows].astype(np.float16)
            eaS[:, chn, C] = np.float16(1.0)

        # madj[r, t, j] = (MASKV if adj==0 else 0) + 0.2*q_j,
        # for i = h*IPC + t*128 + r
        adjh = edge_adj[b, h * IPC : (h + 1) * IPC, :]
        madj = np.where(adjh > 0, 0.0, MASKV) + 0.2 * q[None, :]
        madj = madj.astype(np.float16).reshape(NTILE, 128, E).transpose(1, 0, 2)

        im = {
            "wident": wident,
            "eaS": np.ascontiguousarray(eaS.reshape(128, 4 * (C + 1))),
            "madj": np.ascontiguousarray(madj.reshape(128, NTILE * E)),
        }
        # T parts: tab{i} = chunks 4i..4i+4, laid out [128, 4*E]
        Tq = T.reshape(NCH, 128, E)
        for i in range(NCH // 4):
            im[f"tab{i}"] = np.ascontiguousarray(
                Tq[4 * i : 4 * i + 4].transpose(1, 0, 2).reshape(128, 4 * E)
            )
        # W per out tile: wint{t}[p, (q m)] = Wt[q*128+p, t*128+m]
        Wq = Wt.reshape(NCH, 128, IPC)
        for tt in range(NTILE):
            im[f"wint{tt}"] = np.ascontiguousarray(
                Wq[:, :, tt * 128 : (tt + 1) * 128]
                .transpose(1, 0, 2)
                .reshape(128, NCH * 128)
            )
        in_maps.append(im)
    return in_maps


def kernel(edge_attr, edge_adj, e_max=None, mask=None, W_2=None, U_2=None, yita=None):
    nc = _get_nc()
    in_maps = _host_prep(edge_attr, edge_adj, W_2, U_2, yita)
    res = run_bass_kernel_spmd(nc, in_maps, core_ids=list(range(NCORE)))
    out = np.empty((BSZ, E, C), dtype=np.float32)
    for core in range(NCORE):
        b, h = divmod(core, 2)
        out[b, h * IPC : (h + 1) * IPC, :] = res.results[core]["out"].astype(
            np.float32
        )
    return out


# revision 20
# speedup vs baseline: 1.5973x; 1.1005x over previous
"""GAT neighbor-aggregation kernel for Trainium2, 8-core data-parallel.

Math (per batch b):
  vu = ea @ U2 ; iv = ea @ W2
  logits[i,j] = sum_c yita_c * leaky_relu(vu[i,c] + iv[j,c], 0.2)
  alpha = softmax_j(where(adj>0, logits, -1e12))
  out = leaky_relu(alpha @ ea, 0.2)

Device decomposition (quantized-interpolation table matmul):
  leaky_relu(v) = 0.8*relu(v) + 0.2*v, so with s = vu*|yita|, t = iv*|yita|,
  w_c = 0.8*sign(yita_c):
    logits[i,j] = 0.2*p_i + 0.2*q_j + sum_c w_c * relu(s[i,c] + t[j,c])
  (p_i dropped: softmax row constant).  relu(s+t) is piecewise linear in s,
  so with per-(core,c) uniform levels sigma_{c,0..L-1} over [min_i s, max_i s]:
    relu(s_ic + t_jc) = (1-u)*relu(sigma_l + t_jc) + u*relu(sigma_{l+1} + t_jc)
  EXACTLY unless the cell straddles the kink -t_jc (error <= cell/4 there;
  measured end-to-end rel error ~9e-3 at L=16, budget 2e-2).  Therefore
    R[i,j] ~= sum_{(c,l)} W[(c,l),i] * T[(c,l),j]
  one dense matmul with contraction K = 64*L = 1024 (8 chained 128-row
  matmuls per 128-i output tile, ~215ns each).  W (interpolation weights,
  2 nonzeros per c per column) and T (tables relu(sigma+t)) are
  host-precomputed from the O(e*c) quantities; the O(e^2) work runs on the
  tensor engine at full 128-wide utilization instead of elementwise-bound.
  Uploads are split across the three DMA queues and chunk-pipelined so the
  PE starts as soon as the first W/T pieces land.

  The adjacency mask (+ the 0.2*q_j column bias) is one more accumulated
  matmul adding -60000*(1-adj)+0.2*q_j (identity lhsT, fp16 rhs), so
  exp(masked logit) flushes to 0.  Softmax runs without max-subtraction
  (|logits| < 8, fp16-exp safe): exp in 256-col chunks (scalar engine,
  PSUM->SBUF fp16), transpose per 128 cols (PE), copy (vector engine),
  alpha @ eaS matmul (PE, ones column = denominator), then
  out = Prelu(P * 1/denom, 0.2) in fp16.

Sharding: core = 2*b + h handles batch b, query rows i in [256h, 256h+256).
"""

import numpy as np
from contextlib import ExitStack

import concourse.bass as bass
import concourse.tile as tile
from concourse import bacc, mybir
from concourse.bass_utils import run_bass_kernel_spmd

F32 = mybir.dt.float32
F16 = mybir.dt.float16
OP = mybir.AluOpType

BSZ, E, C = 4, 512, 64
NCORE = 8
IPC = E // 2          # 256 query rows per core
NTILE = IPC // 128    # 2 logits tiles of 128 i-rows
LVL = 16              # interpolation levels per channel
K = C * LVL           # table contraction size (1024)
NCH = K // 128        # 8 contraction chunks of 128
N_WARM = 56           # PE warmup matmuls issued while input DMAs are in flight
MASKV = -60000.0      # mask add value; exp(-60000) == 0 in fp16/fp32


def _build_program():
    nc = bacc.Bacc(
        "TRN2",
        target_bir_lowering=False,
        debug=False,
        enable_asserts=False,
        num_devices=NCORE,
    )
    # T in 4 parts of 2 chunks, W0 in 2 parts of 4 chunks: the PE starts on
    # (W0a, tab0) while the rest is on the wire
    tab_aps = [
        nc.dram_tensor(f"tab{i}", [128, 2 * E], F16, kind="ExternalInput").ap()
        for i in range(NCH // 2)
    ]
    w0_aps = [
        nc.dram_tensor(f"w0{i}", [128, 4 * 128], F16, kind="ExternalInput").ap()
        for i in range(2)
    ]
    w1_ap = nc.dram_tensor("w1", [128, NCH * 128], F16, kind="ExternalInput").ap()
    wident_ap = nc.dram_tensor("wident", [128, 128], F16, kind="ExternalInput").ap()
    eaS_ap = nc.dram_tensor("eaS", [128, 4 * (C + 1)], F16, kind="ExternalInput").ap()
    madj_ap = nc.dram_tensor("madj", [128, NTILE * E], F16, kind="ExternalInput").ap()
    out_ap = nc.dram_tensor("out", [IPC, C], F16, kind="ExternalOutput").ap()

    with tile.TileContext(nc) as tc:
        with ExitStack() as ctx:
            singles = ctx.enter_context(tc.tile_pool(name="singles", bufs=1))
            ps_logits = ctx.enter_context(
                tc.tile_pool(name="ps_logits", bufs=2, space="PSUM")
            )
            ps_tp = ctx.enter_context(tc.tile_pool(name="ps_tp", bufs=2, space="PSUM"))
            ps_fm = ctx.enter_context(tc.tile_pool(name="ps_fm", bufs=2, space="PSUM"))
            small = ctx.enter_context(tc.tile_pool(name="small", bufs=6))
            epool = ctx.enter_context(tc.tile_pool(name="epool", bufs=4))
            atpool = ctx.enter_context(tc.tile_pool(name="atpool", bufs=4))

            # ---- PE warmup: no input deps, runs during the DMA fill ----
            warm_sb = singles.tile([128, C], F16, tag="warm")
            nc.vector.memset(warm_sb[:], 0.0)
            warm_ps = ps_fm.tile([C, C], F32, tag="fm")
            for _ in range(N_WARM):
                nc.tensor.matmul(warm_ps[:], lhsT=warm_sb[:, 0:C], rhs=warm_sb[:])

            # ---- input DMAs, balanced across the three DMA queues; the
            # pieces gating the first table matmuls (W0a, tab0) go first ----
            w0 = []
            for i in range(2):
                wt = singles.tile([128, 4, 128], F16, tag=f"w0{i}")
                nc.scalar.dma_start(wt[:], w0_aps[i].rearrange("p (q m) -> p q m", q=4))
                w0.append(wt)
            tabs = []
            tab_eng = [nc.sync, nc.gpsimd, nc.sync, nc.gpsimd]
            for i in range(NCH // 2):
                tb = singles.tile([128, 2, E], F16, tag=f"tab{i}")
                tab_eng[i].dma_start(
                    tb[:], tab_aps[i].rearrange("p (q j) -> p q j", q=2)
                )
                tabs.append(tb)
            w1 = singles.tile([128, NCH, 128], F16, tag="w1")
            nc.scalar.dma_start(w1[:], w1_ap.rearrange("p (q m) -> p q m", q=NCH))
            madj_sb = singles.tile([128, NTILE, E], F16, tag="madj")
            nc.gpsimd.dma_start(
                madj_sb[:], madj_ap.rearrange("p (t j) -> p t j", t=NTILE)
            )
            ident_sb = singles.tile([128, 128], F16, tag="ident")
            nc.sync.dma_start(ident_sb[:], wident_ap[:])
            eaS = singles.tile([128, 4, C + 1], F16, tag="eaS")
            nc.gpsimd.dma_start(eaS[:], eaS_ap.rearrange("p (ch c) -> p ch c", ch=4))

            def wchunk(t, q):
                if t == 1:
                    return w1[:, q, :]
                return w0[q // 4][:, q % 4, :]

            # ---- logits: 8 chained dense matmuls per 128-i tile ----
            for t in range(NTILE):
                logits_ps = ps_logits.tile([128, E], F32, tag="logits")
                for q in range(NCH):
                    nc.tensor.matmul(
                        logits_ps[:],
                        lhsT=wchunk(t, q),
                        rhs=tabs[q // 2][:, q % 2, :],
                        start=(q == 0),
                        stop=False,
                    )
                # mask + column bias: logits += -60000*(1-adj) + 0.2*q_j
                nc.tensor.matmul(
                    logits_ps[:],
                    lhsT=ident_sb[:],
                    rhs=madj_sb[:, t, :],
                    start=False,
                    stop=True,
                    skip_group_check=True,
                )
                # softmax numerator (no max-sub): exp per 256-col chunk, then
                # transpose/copy/fm per 128-col chunk; the ones column of eaS
                # yields the denominator through the fm matmul
                fm_ps = ps_fm.tile([128, C + 1], F32, tag="fm")
                for hh in range(2):
                    e_h = epool.tile([128, 256], F16, tag="esb")
                    nc.scalar.activation(
                        e_h[:], logits_ps[:, hh * 256 : (hh + 1) * 256],
                        mybir.ActivationFunctionType.Exp, bias=0.0, scale=1.0,
                    )
                    for cc in range(2):
                        ch = hh * 2 + cc
                        tp = ps_tp.tile([128, 128], F16, tag="tp")
                        nc.tensor.transpose(
                            tp[:], e_h[:, cc * 128 : (cc + 1) * 128], ident_sb
                        )
                        aT = atpool.tile([128, 128], F16, tag="aT")
                        nc.vector.tensor_copy(aT[:], tp[:])
                        nc.tensor.matmul(
                            fm_ps[:],
                            lhsT=aT[:],
                            rhs=eaS[:, ch, :],
                            start=(ch == 0),
                            stop=(ch == 3),
                        )
                # out = leaky_relu(P / denom) = prelu(P * rec, 0.2), rec > 0
                rec = small.tile([128, 1], F32, tag="rec")
                nc.vector.reciprocal(rec[:], fm_ps[:, C : C + 1])
                out_sb = small.tile([128, C], F16, tag="outsb")
                nc.scalar.activation(
                    out_sb[:], fm_ps[:, 0:C], mybir.ActivationFunctionType.Prelu,
                    bias=0.0, scale=rec[:], alpha=0.2,
                )
                nc.sync.dma_start(out_ap[t * 128 : (t + 1) * 128, :], out_sb[:])

    nc.finalize()
    return nc


_NC = None


def _get_nc():
    global _NC
    if _NC is None:
        _NC = _build_program()
    return _NC


def _host_prep(edge_attr, edge_adj, W_2, U_2, yita):
    edge_attr = np.asarray(edge_attr, dtype=np.float32)
    edge_adj = np.asarray(edge_adj)
    W_2 = np.asarray(W_2, dtype=np.float32)
    U_2 = np.asarray(U_2, dtype=np.float32)
    yita = np.asarray(yita, dtype=np.float32)

    y = yita[:, 0]
    ay = np.abs(y)
    w = (0.8 * np.sign(y)).astype(np.float32)
    wident = np.eye(128, dtype=np.float16)

    in_maps = []
    for core in range(NCORE):
        b, h = divmod(core, 2)
        ea = edge_attr[b]                      # [E, C]
        vu = ea @ U_2
        iv = ea @ W_2
        s = vu * ay[None, :]                   # [E, C]
        t = iv * ay[None, :]                   # [E, C]
        q = iv @ y                             # [E]

        sh = s[h * IPC : (h + 1) * IPC]        # [IPC, C]
        lo, hi = sh.min(0), sh.max(0)          # [C]
        span = np.maximum(hi - lo, 1e-6)
        sig = lo[None, :] + span[None, :] * (
            np.arange(LVL, dtype=np.float32)[:, None] / (LVL - 1)
        )
        # tables T[(c,l), j] = relu(sig[l,c] + t[j,c]), contraction-major
        T = np.maximum(sig.T[:, :, None] + t.T[:, None, :], 0.0)  # [C, LVL, E]
        T = T.reshape(K, E).astype(np.float16)

        # interpolation weights W[(c,l), i_local]
        delta = span / (LVL - 1)
        u = (sh - lo[None, :]) / delta[None, :]
        l0 = np.clip(np.floor(u).astype(np.int64), 0, LVL - 2)
        frac = (u - l0).astype(np.float32)
        Wt = np.zeros((C, LVL, IPC), dtype=np.float32)
        ii = np.arange(IPC)
        for c in range(C):
            Wt[c, l0[:, c], ii] += w[c] * (1.0 - frac[:, c])
            Wt[c, l0[:, c] + 1, ii] += w[c] * frac[:, c]
        Wt = Wt.reshape(K, IPC).astype(np.float16)

        # eaS[:, ch, 0:C] = ea in chunk layout, col C = 1 (denominator)
        eaS = np.empty((128, 4, C + 1), dtype=np.float16)
        for chn in range(4):
            rows = slice(chn * 128, (chn + 1) * 128)
            eaS[:, chn, 0:C] = ea[rows].astype(np.float16)
            eaS[:, chn, C] = np.float16(1.0)

        # madj[r, t, j] = (MASKV if adj==0 else 0) + 0.2*q_j
        adjh = edge_adj[b, h * IPC : (h + 1) * IPC, :]
        madj = np.where(adjh > 0, 0.0, MASKV) + 0.2 * q[None, :]
        madj = madj.astype(np.float16).reshape(NTILE, 128, E).transpose(1, 0, 2)

        im = {
            "wident": wident,
            "eaS": np.ascontiguousarray(eaS.reshape(128, 4 * (C + 1))),
            "madj": np.ascontiguousarray(madj.reshape(128, NTILE * E)),
        }
        Tq = T.reshape(NCH, 128, E)
        for i in range(NCH // 2):
            im[f"tab{i}"] = np.ascontiguousarray(
                Tq[2 * i : 2 * i + 2].transpose(1, 0, 2).reshape(128, 2 * E)
            )
        Wq = Wt.reshape(NCH, 128, IPC)
        for i in range(2):
            im[f"w0{i}"] = np.ascontiguousarray(
                Wq[4 * i : 4 * i + 4, :, 0:128].transpose(1, 0, 2).reshape(128, 4 * 128)
            )
        im["w1"] = np.ascontiguousarray(
            Wq[:, :, 128:256].transpose(1, 0, 2).reshape(128, NCH * 128)
        )
        in_maps.append(im)
    return in_maps


def kernel(edge_attr, edge_adj, e_max=None, mask=None, W_2=None, U_2=None, yita=None):
    nc = _get_nc()
    in_maps = _host_prep(edge_attr, edge_adj, W_2, U_2, yita)
    res = run_bass_kernel_spmd(nc, in_maps, core_ids=list(range(NCORE)))
    out = np.empty((BSZ, E, C), dtype=np.float32)
    for core in range(NCORE):
        b, h = divmod(core, 2)
        out[b, h * IPC : (h + 1) * IPC, :] = res.results[core]["out"].astype(
            np.float32
        )
    return out
